# revision 19
# baseline (speedup 1.0000x reference)
"""Trainium2 Bass kernel for nn_Decoder (LSTM decoder with SE/HP MLP heads).

Strategy: pure data parallelism over batch (2048 -> 8 cores x 256).
Feature-major on-chip layout ([feature, batch]); weights stationary, batch
on the matmul moving dim.

v2 changes vs the bf16 baseline:
  - The recurrent h @ w_hh.T contraction runs in fp8e4 with
    perf_mode=DoubleRow: 2 contraction elements per PE cell -> K=256 per
    matmul, halving the h-part matmul count (32 DR MMs/step vs 64 bf16).
    h is stored fp8; w_hh is pre-quantized fp8 on the host.  Accuracy was
    validated on CPU: end-to-end rel err ~3.3e-3 (tolerance 2e-2).
  - The K=18 SE x-part matmuls are packed 4-at-a-time into distinct 32-row
    PE row-groups via tile_position; relu_u is replicated into 4 partition
    groups by a single widened K=2 matmul (a1_ext) + one fused DVE
    relu+bias, so the 4 concurrent row-group MMs each have a properly
    based rhs.  Gate biases stay folded as two bf16 hi/lo contraction rows
    whose moving values (==1.0) are produced by the same relu trick.
  - Gate activations output bf16; the LSTM cell state c is bf16; all cell
    element-wise ops run 2x on DVE (16-bit packing).  (GpSimd offload was
    tried and reverted: Pool ops grab the SBUF port pair that DVE
    tensor_tensor needs, serializing instead of overlapping.)
    h = o*tanh(c) writes fp8 directly.
  - The HP head v-matmul also runs fp8 DoubleRow (2 MMs); the lp carry-add
    is folded into the p-head PSUM group via an f32r identity matmul, and
    the SE-head u-matmul reads the traj slice through an f32r bitcast, so
    the whole lp chain has no DVE copies on it.
  - DR fills for the next step are emitted in dependency-sorted blocks
    (h-half-0 gated, then h-half-1 gated) so the PE always has prefill
    work while the sigmoid stream and cell math run.

Per step (32 sequential steps):
  gates = sum_j whh_fp8[j] (x)DR h_fp8[j]  + w2t_ext.T @ relu_u4   [2048,256]
  i,f,o = sigmoid(gates), g = tanh(gates)  (ACT, bf16 out)
  c = f*c + i*g (DVE bf16 2x); h = o*tanh(c) (ACT tanh + DVE, fp8 out)
  v = relu(b1_fp8 (x)DR h + c1hp); p = hpw2.T @ v; lp = sigmoid(p+lp+b)
  u = a1_ext.T @ lp; relu_u4 = relu(u + c1se4)  (4 partition-group copies)
"""

import json

import numpy as np
import ml_dtypes
from contextlib import ExitStack

import concourse.bass as bass
import concourse.mybir as mybir
import concourse.tile as tile
from concourse.bass import ts


def _fix_multiwait(bir_bytes: bytes) -> bytes:
    """Hoist excess sync waits onto injected EventSemaphore carriers
    (HW cap: 2 waits on EventSemaphore, 1 elsewhere; the Tile end-of-kernel
    drain can exceed this and the compiler rejects it)."""
    bir = json.loads(bir_bytes)
    for fn in bir.get("functions", []):
        for blk in fn.get("blocks", []):
            insts = blk.get("instructions")
            if not insts:
                continue
            out = []
            for inst in insts:
                si = inst.get("sync_info")
                waits = (si or {}).get("on_wait") or []
                cap = 2 if inst.get("opcode") == "EventSemaphore" else 1
                if len(waits) > cap:
                    excess, keep = waits[:-cap], waits[-cap:]
                    si["on_wait"] = keep
                    for i in range(0, len(excess), 2):
                        out.append({
                            "debug": inst.get("debug", 0),
                            "engine": inst["engine"],
                            "ins": [],
                            "name": f"{inst['name']}_xw{i}",
                            "opcode": "EventSemaphore",
                            "outs": [],
                            "sync_info": {"on_update": [], "on_wait": excess[i : i + 2]},
                        })
                out.append(inst)
            blk["instructions"] = out
    return json.dumps(bir).encode()


BF16 = ml_dtypes.bfloat16
F8 = ml_dtypes.float8_e4m3
F32 = np.float32

SEQ = 32
B = 2048
H = 512
E = 512
HID = 16
NCORES = 8
BL = B // NCORES  # 256 local batch
NG = 4 * H  # 2048 gate features
BN_EPS = 1e-5

_CACHE: dict = {}


def _build_nc(repeats: int = 1):
    nc = bass.Bass()
    dt = mybir.dt
    ACTF = mybir.ActivationFunctionType
    ALU = mybir.AluOpType
    DR = mybir.MatmulPerfMode.DoubleRow

    # --- DRAM tensors (per-core inputs; weights replicated across cores) ---
    whh_d = nc.dram_tensor("whhdr", [2, 128, 2 * NG], dt.float8e4, kind="ExternalInput")
    w2t4_d = nc.dram_tensor("w2t4", [128, NG], dt.bfloat16, kind="ExternalInput")
    a1e_d = nc.dram_tensor("a1e", [2, 128], dt.float32r, kind="ExternalInput")
    id2_d = nc.dram_tensor("id2", [2, 2], dt.float32r, kind="ExternalInput")
    c1se4_d = nc.dram_tensor("c1se4", [128, 1], dt.float32, kind="ExternalInput")
    b1dr_d = nc.dram_tensor("b1dr", [2, 128, 2 * HID], dt.float8e4, kind="ExternalInput")
    c1hp_d = nc.dram_tensor("c1hp", [HID, 1], dt.float32, kind="ExternalInput")
    hpw2_d = nc.dram_tensor("hpw2", [HID, 2], dt.bfloat16, kind="ExternalInput")
    hpb2_d = nc.dram_tensor("hpb2", [2, 1], dt.float32, kind="ExternalInput")
    zero_d = nc.dram_tensor("zero128", [128, BL], dt.bfloat16, kind="ExternalInput")
    lp0f_d = nc.dram_tensor("lp0f", [2, BL], dt.float32r, kind="ExternalInput")
    h0_d = nc.dram_tensor("h0f8", [2, 128, 2 * BL], dt.float8e4, kind="ExternalInput")
    c0_d = nc.dram_tensor("c0b", [2, 128, 2 * BL], dt.bfloat16, kind="ExternalInput")
    traj_d = nc.dram_tensor("traj", [2, SEQ, BL], dt.float32, kind="ExternalOutput")

    with tile.TileContext(nc) as tc:
        with ExitStack() as ctx:
            singles = ctx.enter_context(tc.tile_pool(name="singles", bufs=1))
            gpool = ctx.enter_context(tc.tile_pool(name="gates", bufs=2))
            tpool = ctx.enter_context(tc.tile_pool(name="temps", bufs=3))
            psg = ctx.enter_context(tc.tile_pool(name="psg", bufs=7, space="PSUM"))
            pss = ctx.enter_context(tc.tile_pool(name="pss", bufs=1, space="PSUM"))

            # persistent weights
            whh = []
            for j in range(2):
                wt = singles.tile([128, 2 * NG], dt.float8e4, tag=f"whh{j}", name=f"whh{j}")
                nc.sync.dma_start(out=wt, in_=whh_d[j, :, :])
                whh.append(wt)
            w2t4 = singles.tile([128, NG], dt.bfloat16)
            nc.sync.dma_start(out=w2t4, in_=w2t4_d[:, :])
            a1e = singles.tile([2, 128], dt.float32r)
            nc.sync.dma_start(out=a1e, in_=a1e_d[:, :])
            id2 = singles.tile([2, 2], dt.float32r)
            nc.sync.dma_start(out=id2, in_=id2_d[:, :])
            c1se4 = singles.tile([128, 1], dt.float32)
            nc.sync.dma_start(out=c1se4, in_=c1se4_d[:, :])
            b1dr = []
            for j in range(2):
                bt = singles.tile([128, 2 * HID], dt.float8e4, tag=f"b1dr{j}", name=f"b1dr{j}")
                nc.sync.dma_start(out=bt, in_=b1dr_d[j, :, :])
                b1dr.append(bt)
            c1hp = singles.tile([HID, 1], dt.float32)
            nc.sync.dma_start(out=c1hp, in_=c1hp_d[:, :])
            hpw2 = singles.tile([HID, 2], dt.bfloat16)
            nc.sync.dma_start(out=hpw2, in_=hpw2_d[:, :])
            hpb2 = singles.tile([2, 1], dt.float32)
            nc.sync.dma_start(out=hpb2, in_=hpb2_d[:, :])
            zero128 = singles.tile([128, BL], dt.bfloat16)
            nc.sync.dma_start(out=zero128, in_=zero_d[:, :])

            # persistent state
            traj = singles.tile([2, SEQ * BL], dt.float32)
            lp0f = singles.tile([2, BL], dt.float32r)
            nc.sync.dma_start(out=lp0f, in_=lp0f_d[:, :])
            # h ping-pong: step t reads hb2[t%2], writes hb2[(t+1)%2]; fp8
            hb2, cst = {0: [], 1: []}, []
            for j in range(2):
                t_b = singles.tile([128, 2 * BL], dt.float8e4, tag=f"hbA{j}", name=f"hbA{j}")
                nc.sync.dma_start(out=t_b, in_=h0_d[j, :, :])
                hb2[0].append(t_b)
                t_b2 = singles.tile([128, 2 * BL], dt.float8e4, tag=f"hbB{j}", name=f"hbB{j}")
                hb2[1].append(t_b2)
                t_c = singles.tile([128, 2 * BL], dt.bfloat16, tag=f"c{j}", name=f"c{j}")
                nc.sync.dma_start(out=t_c, in_=c0_d[j, :, :])
                cst.append(t_c)

            whh3 = [w.rearrange("p (i c) -> p i c", i=2) for w in whh]
            b1dr3 = [w.rearrange("p (i c) -> p i c", i=2) for w in b1dr]

            pairs: dict = {}

            def dr(t, p, half, j):
                """fp8 DoubleRow h-matmul for gate m-tile m=2p+half,
                contraction pair j (h features 256j..256j+255).  The first
                matmul of the pair opens the whole-bank accumulation group;
                the group is closed by the two x-part matmuls (xmm), so all
                4 DR matmuls are independent of the lp chain and prefill."""
                ps = pairs.get((t, p))
                start = ps is None
                if start:
                    ps = psg.tile([128, 2 * BL], dt.float32, tag="gp", name=f"gp{t}_{p}")
                    pairs[(t, p)] = ps
                m = 2 * p + half
                nc.tensor.matmul(
                    ps[:, ts(half, BL)],
                    whh3[j][:, :, ts(m, 128)],
                    hb2[t % 2][j].rearrange("p (i b) -> p i b", i=2),
                    start=start, stop=False, perf_mode=DR,
                )

            def xmm(t, p, half, ru):
                """Close m-tile m=2p+half with the SE x-part (K=18, biases
                folded as hi/lo rows vs the ==1.0 rows of relu_u4).  Row
                group g=m//4 so 4 consecutive closes run concurrently."""
                ps = pairs[(t, p)]
                m = 2 * p + half
                g = m // 4
                nc.tensor.matmul(
                    ps[:, ts(half, BL)],
                    w2t4[32 * g : 32 * g + 18, ts(m, 128)],
                    ru[32 * g : 32 * g + 18, :],
                    start=False, stop=(half == 1), tile_position=(32 * g, 0),
                )

            def act(t, p):
                ps = pairs.pop((t, p))
                func = ACTF.Tanh if p in (4, 5) else ACTF.Sigmoid
                gs = gpool.tile(
                    [128, 2 * BL], dt.bfloat16, tag=f"gate{p}", name=f"gate{t}_{p}"
                )
                nc.scalar.activation(gs, ps, func)
                return gs

            def elem(t, j, gs):
                """LSTM cell update for feature-half j (bf16 on DVE at 2x,
                i*g on GpSimd); writes c in place and the NEXT ping-pong h
                buffer directly in fp8."""
                i_t, f_t, g_t, o_t = gs[0 + j], gs[2 + j], gs[4 + j], gs[6 + j]
                t_ig = tpool.tile([128, 2 * BL], dt.bfloat16, tag="tig", name=f"tig{t}_{j}")
                nc.vector.tensor_mul(t_ig, i_t, g_t)
                t_fc = tpool.tile([128, 2 * BL], dt.bfloat16, tag="tfc", name=f"tfc{t}_{j}")
                nc.vector.tensor_mul(t_fc, f_t, cst[j])
                nc.vector.tensor_add(cst[j], t_fc, t_ig)
                t_tc = tpool.tile([128, 2 * BL], dt.bfloat16, tag="ttc", name=f"ttc{t}_{j}")
                nc.scalar.activation(t_tc, cst[j], ACTF.Tanh)
                nc.vector.tensor_mul(hb2[(t + 1) % 2][j], o_t, t_tc)

            def body():
                # prologue: relu_u4 for t=0 + full prefills of pairs 0,2,4,6,1,3
                u_ps = pss.tile([128, BL], dt.float32, tag="small", name="u_0")
                nc.tensor.matmul(u_ps, a1e, lp0f, start=True, stop=True)
                ru = tpool.tile([128, BL], dt.bfloat16, tag="ru", name="ru0", bufs=2)
                nc.vector.scalar_tensor_tensor(
                    ru, u_ps, c1se4, zero128, ALU.add, ALU.max
                )
                for p in (0, 2, 4, 6, 1, 3):
                    dr(0, p, 0, 0)
                    dr(0, p, 1, 0)
                    dr(0, p, 0, 1)
                    dr(0, p, 1, 1)

                for t in range(SEQ):
                    lp_f = (
                        lp0f[:, :] if t == 0
                        else traj[:2, ts(t - 1, BL)].bitcast(dt.float32r)
                    )
                    nxt = t + 1 if t + 1 < SEQ else None
                    gs = {}

                    # ---- even pairs: x-closes first so the sigmoid stream
                    # starts as soon as relu_u4 lands ----
                    for p in (0, 2, 4, 6):
                        xmm(t, p, 0, ru)
                    for p in (0, 2, 4, 6):
                        xmm(t, p, 1, ru)
                    for p in (0, 4, 2, 6):
                        gs[p] = act(t, p)
                    elem(t, 0, gs)

                    # in-step DR fills for pairs 5,7 keep PE busy under the
                    # even-σ stream; then x-odds (ready: only need relu_u)
                    for p in (5, 7):
                        dr(t, p, 0, 0)
                        dr(t, p, 1, 0)
                        dr(t, p, 0, 1)
                        dr(t, p, 1, 1)
                    for p in (1, 3, 5, 7):
                        xmm(t, p, 0, ru)
                    for p in (1, 3, 5, 7):
                        xmm(t, p, 1, ru)
                    for p in (1, 5, 3, 7):
                        gs[p] = act(t, p)

                    # h0-gated PE block: v j0 + ALL j0 prefills run while the
                    # odd-σ stream / cell half 1 / lp chain proceed, keeping
                    # the PE warm (no >3.4us idle -> no HAM re-throttle)
                    nh = hb2[(t + 1) % 2]
                    v_ps = pss.tile([HID, BL], dt.float32, tag="small", name=f"v{t}")
                    nc.tensor.matmul(
                        v_ps, b1dr3[0], nh[0].rearrange("p (i b) -> p i b", i=2),
                        start=True, stop=False, perf_mode=DR,
                    )
                    if nxt is not None:
                        for p in (0, 2, 4, 6, 1, 3):
                            dr(nxt, p, 0, 0)
                            dr(nxt, p, 1, 0)

                    elem(t, 1, gs)

                    # ---- tail: h1-gated block, then lp chain ----
                    nc.tensor.matmul(
                        v_ps, b1dr3[1], nh[1].rearrange("p (i b) -> p i b", i=2),
                        start=False, stop=True, perf_mode=DR,
                    )
                    if nxt is not None:
                        for p in (0, 2, 4, 6):
                            dr(nxt, p, 0, 1)
                            dr(nxt, p, 1, 1)
                    r_hp = tpool.tile([HID, BL], dt.bfloat16, tag="rhp", name=f"rhp{t}")
                    nc.vector.scalar_tensor_tensor(
                        r_hp, v_ps, c1hp, zero128[:HID, :], ALU.add, ALU.max
                    )
                    # p_ps accumulates lp (via f32r identity matmul) + HP head
                    p_ps = pss.tile([2, BL], dt.float32, tag="small", name=f"p{t}")
                    nc.tensor.matmul(p_ps, id2, lp_f, start=True, stop=False)
                    nc.tensor.matmul(p_ps, hpw2, r_hp, start=False, stop=True)
                    if nxt is not None:
                        for p in (1, 3):
                            dr(nxt, p, 0, 1)
                            dr(nxt, p, 1, 1)
                    nc.scalar.activation(
                        traj[:2, ts(t, BL)].bitcast(dt.float32r),
                        p_ps, ACTF.Sigmoid, bias=hpb2,
                    )
                    if nxt is not None:
                        u_ps = pss.tile([128, BL], dt.float32, tag="small", name=f"u{nxt}")
                        nc.tensor.matmul(
                            u_ps, a1e, traj[:2, ts(t, BL)].bitcast(dt.float32r),
                            start=True, stop=True,
                        )
                        ru = tpool.tile([128, BL], dt.bfloat16, tag="ru", name=f"ru{nxt}", bufs=2)
                        nc.vector.scalar_tensor_tensor(
                            ru, u_ps, c1se4, zero128, ALU.add, ALU.max
                        )

                if repeats > 1:
                    # restore loop-carried lp state for the next timing rep
                    nc.vector.tensor_copy(lp0f, traj[:2, ts(SEQ - 1, BL)])

            if repeats == 1:
                body()
            else:
                with tc.For_i(0, repeats, 1):
                    body()

            nc.sync.dma_start(
                out=traj_d[:, :, :].rearrange("p t b -> p (t b)"), in_=traj[:2, :]
            )
    patched = _fix_multiwait(nc.to_json_bytes())
    nc.to_json_bytes = lambda: patched
    return nc


def _pack_half(x_t):
    # [512, BL] feature-major -> [2, 128, 2*BL]: tile j holds feature-tiles
    # 2j (cols 0:BL) and 2j+1 (cols BL:2BL)
    xr = x_t.reshape(4, 128, BL)
    return np.stack(
        [np.concatenate([xr[2 * j], xr[2 * j + 1]], axis=1) for j in range(2)]
    )


def _host_prep(inputs):
    f = lambda k: np.asarray(inputs[k], dtype=np.float64)
    se_w1, se_b1 = f("se_w1"), f("se_b1")
    se_g, se_bt, se_m, se_v = f("se_g"), f("se_bt"), f("se_m"), f("se_v")
    se_w2, se_b2 = f("se_w2"), f("se_b2")
    w_ih, w_hh, b_ih, b_hh = f("w_ih"), f("w_hh"), f("b_ih"), f("b_hh")
    hp_w1, hp_b1 = f("hp_w1"), f("hp_b1")
    hp_g, hp_bt, hp_m, hp_v = f("hp_g"), f("hp_bt"), f("hp_m"), f("hp_v")
    hp_w2, hp_b2 = f("hp_w2"), f("hp_b2")

    s_se = se_g / np.sqrt(se_v + BN_EPS)
    a1 = (se_w1 * s_se[None, :]).astype(F32)  # [2, 16]
    c1_se = ((se_b1 - se_m) * s_se + se_bt).astype(F32)
    s_hp = hp_g / np.sqrt(hp_v + BN_EPS)
    b1 = (hp_w1 * s_hp[None, :]).astype(F32)  # [512, 16]
    c1_hp = ((hp_b1 - hp_m) * s_hp + hp_bt).astype(F32)

    w2t = (se_w2 @ w_ih.T).astype(F32)  # [16, 2048]
    b_eff = (b_ih + b_hh + w_ih @ se_b2).astype(F32)  # [2048]
    b_hi = b_eff.astype(BF16).astype(F32)
    b_lo = (b_eff - b_hi).astype(F32)
    w2t_ext = np.concatenate(
        [w2t, b_hi[None, :], b_lo[None, :]], axis=0
    ).astype(BF16)  # [18, 2048]

    # 4 partition-group replicas for the row-tiled x-part
    w2t4 = np.zeros((128, NG), dtype=BF16)
    a1e = np.zeros((2, 128), dtype=F32)
    c1se4 = np.zeros((128, 1), dtype=F32)
    for g in range(4):
        w2t4[32 * g : 32 * g + 18, :] = w2t_ext
        a1e[:, 32 * g : 32 * g + 16] = a1
        c1se4[32 * g : 32 * g + 16, 0] = c1_se
        c1se4[32 * g + 16 : 32 * g + 18, 0] = 1.0  # the ==1.0 bias rows

    # DoubleRow weight pairs: whhdr[j][p, i*NG+col] = w_hh.T[256j+128i+p, col]
    wT = np.ascontiguousarray(w_hh.T)  # [512, 2048]
    whhdr = wT.reshape(2, 2, 128, NG).transpose(0, 2, 1, 3).reshape(2, 128, 2 * NG)
    whhdr = whhdr.astype(F32).astype(F8)
    b1dr = b1.reshape(2, 2, 128, HID).transpose(0, 2, 1, 3).reshape(2, 128, 2 * HID)
    b1dr = b1dr.astype(F8)

    rep = {
        "whhdr": np.ascontiguousarray(whhdr),
        "w2t4": np.ascontiguousarray(w2t4),
        "a1e": np.ascontiguousarray(a1e),
        "id2": np.eye(2, dtype=F32),
        "c1se4": c1se4,
        "b1dr": np.ascontiguousarray(b1dr),
        "c1hp": c1_hp.reshape(HID, 1),
        "hpw2": np.ascontiguousarray(hp_w2.astype(F32).astype(BF16)),
        "hpb2": hp_b2.astype(F32).reshape(2, 1),
        "zero128": np.zeros((128, BL), dtype=BF16),
    }

    last_pos = np.asarray(inputs["last_pos"], dtype=F32)
    h0 = np.asarray(inputs["hh"], dtype=F32)[0]
    c0 = np.asarray(inputs["ch"], dtype=F32)[0]
    in_maps = []
    for c in range(NCORES):
        rows = slice(c * BL, (c + 1) * BL)
        h0t = np.ascontiguousarray(h0[rows].T)  # [512, BL]
        c0t = np.ascontiguousarray(c0[rows].T)
        m = dict(rep)
        lp0t = np.ascontiguousarray(last_pos[rows].T)  # [2, BL]
        m["lp0f"] = lp0t
        m["h0f8"] = _pack_half(h0t).astype(F8)
        m["c0b"] = _pack_half(c0t).astype(BF16)
        in_maps.append(m)
    return in_maps


def _get_runner(repeats: int = 1):
    """Build (once) a persistent jitted SPMD runner over 8 cores."""
    key = ("runner", repeats)
    if key in _CACHE:
        return _CACHE[key]

    import jax
    from jax.sharding import Mesh, PartitionSpec, NamedSharding
    from jax.experimental.shard_map import shard_map
    from concourse import bass2jax, mybir as _mb

    nc = _build_nc(repeats)
    bass2jax.install_neuronx_cc_hook()

    partition_name = nc.partition_id_tensor.name if nc.partition_id_tensor else None
    in_names, out_names, out_avals, zero_shapes = [], [], [], []
    for alloc in nc.m.functions[0].allocations:
        if not isinstance(alloc, _mb.MemoryLocationSet):
            continue
        name = alloc.memorylocations[0].name
        if alloc.kind == "ExternalInput":
            if name != partition_name:
                in_names.append(name)
        elif alloc.kind == "ExternalOutput":
            out_names.append(name)
            shape = tuple(alloc.tensor_shape)
            dtype = _mb.dt.np(alloc.dtype)
            out_avals.append(jax.core.ShapedArray(shape, dtype))
            zero_shapes.append((shape, dtype))
    n_params = len(in_names)
    all_names = in_names + out_names
    if partition_name is not None:
        all_names = all_names + [partition_name]
    donate = tuple(range(n_params, n_params + len(out_names)))

    def _body(*args):
        operands = list(args)
        if partition_name is not None:
            operands.append(bass2jax.partition_id_tensor())
        outs = bass2jax._bass_exec_p.bind(
            *operands,
            out_avals=tuple(out_avals),
            in_names=tuple(all_names),
            out_names=tuple(out_names),
            lowering_input_output_aliases=(),
            sim_require_finite=True,
            sim_require_nnan=True,
            nc=nc,
        )
        return tuple(outs)

    devices = jax.devices()[:NCORES]
    mesh = Mesh(np.asarray(devices), ("core",))
    spec = PartitionSpec("core")
    sharded = jax.jit(
        shard_map(
            _body,
            mesh=mesh,
            in_specs=(spec,) * (n_params + len(out_names)),
            out_specs=(spec,) * len(out_names),
            check_rep=False,
        ),
        donate_argnums=donate,
        keep_unused=True,
    )
    sharding = NamedSharding(mesh, spec)

    def stage(in_maps):
        """device_put concatenated inputs once; reusable across exec() calls."""
        concat = [
            np.concatenate([np.asarray(m[name]) for m in in_maps], axis=0)
            for name in in_names
        ]
        return [jax.device_put(a, sharding) for a in concat]

    def exec_(staged):
        zeros = [
            jax.device_put(np.zeros((NCORES * s[0], *s[1:]), d), sharding)
            for s, d in zero_shapes
        ]
        outs = sharded(*staged, *zeros)
        outs = [np.asarray(o) for o in outs]
        return {
            name: outs[i].reshape(NCORES, *out_avals[i].shape)
            for i, name in enumerate(out_names)
        }

    _CACHE[key] = (stage, exec_)
    return _CACHE[key]


def kernel(**inputs) -> np.ndarray:
    stage, exec_ = _get_runner()
    staged = stage(_host_prep(inputs))
    per_core = exec_(staged)["traj"]  # [8, 2, 32, BL]
    out = per_core.transpose(2, 0, 3, 1).reshape(SEQ, B, 2)
    return np.ascontiguousarray(out.astype(np.float32))


# revision 21
# speedup vs baseline: 1.0279x; 1.0279x over previous
"""Trainium2 Bass kernel for nn_Decoder (LSTM decoder with SE/HP MLP heads).

Strategy: pure data parallelism over batch (2048 -> 8 cores x 256).
Feature-major on-chip layout ([feature, batch]); weights stationary, batch
on the matmul moving dim.

v2 changes vs the bf16 baseline:
  - The recurrent h @ w_hh.T contraction runs in fp8e4 with
    perf_mode=DoubleRow: 2 contraction elements per PE cell -> K=256 per
    matmul, halving the h-part matmul count (32 DR MMs/step vs 64 bf16).
    h is stored fp8; w_hh is pre-quantized fp8 on the host.  Accuracy was
    validated on CPU: end-to-end rel err ~3.3e-3 (tolerance 2e-2).
  - The K=18 SE x-part matmuls are packed 4-at-a-time into distinct 32-row
    PE row-groups via tile_position; relu_u is replicated into 4 partition
    groups by a single widened K=2 matmul (a1_ext) + one fused DVE
    relu+bias, so the 4 concurrent row-group MMs each have a properly
    based rhs.  Gate biases stay folded as two bf16 hi/lo contraction rows
    whose moving values (==1.0) are produced by the same relu trick.
  - Gate activations output bf16; the LSTM cell state c is bf16; all cell
    element-wise ops run 2x on DVE (16-bit packing).  (GpSimd offload was
    tried and reverted: Pool ops grab the SBUF port pair that DVE
    tensor_tensor needs, serializing instead of overlapping.)
    h = o*tanh(c) writes fp8 directly.
  - The HP head v-matmul also runs fp8 DoubleRow (2 MMs); the lp carry-add
    is folded into the p-head PSUM group via an f32r identity matmul, and
    the SE-head u-matmul reads the traj slice through an f32r bitcast, so
    the whole lp chain has no DVE copies on it.
  - DR fills for the next step are emitted in dependency-sorted blocks
    (h-half-0 gated, then h-half-1 gated) so the PE always has prefill
    work while the sigmoid stream and cell math run.

Per step (32 sequential steps):
  gates = sum_j whh_fp8[j] (x)DR h_fp8[j]  + w2t_ext.T @ relu_u4   [2048,256]
  i,f,o = sigmoid(gates), g = tanh(gates)  (ACT, bf16 out)
  c = f*c + i*g (DVE bf16 2x); h = o*tanh(c) (ACT tanh + DVE, fp8 out)
  v = relu(b1_fp8 (x)DR h + c1hp); p = hpw2.T @ v; lp = sigmoid(p+lp+b)
  u = a1_ext.T @ lp; relu_u4 = relu(u + c1se4)  (4 partition-group copies)
"""

import json

import numpy as np
import ml_dtypes
from contextlib import ExitStack

import concourse.bass as bass
import concourse.mybir as mybir
import concourse.tile as tile
from concourse.bass import ts


def _fix_multiwait(bir_bytes: bytes) -> bytes:
    """Hoist excess sync waits onto injected EventSemaphore carriers
    (HW cap: 2 waits on EventSemaphore, 1 elsewhere; the Tile end-of-kernel
    drain can exceed this and the compiler rejects it)."""
    bir = json.loads(bir_bytes)
    for fn in bir.get("functions", []):
        for blk in fn.get("blocks", []):
            insts = blk.get("instructions")
            if not insts:
                continue
            out = []
            for inst in insts:
                si = inst.get("sync_info")
                waits = (si or {}).get("on_wait") or []
                cap = 2 if inst.get("opcode") == "EventSemaphore" else 1
                if len(waits) > cap:
                    excess, keep = waits[:-cap], waits[-cap:]
                    si["on_wait"] = keep
                    for i in range(0, len(excess), 2):
                        out.append({
                            "debug": inst.get("debug", 0),
                            "engine": inst["engine"],
                            "ins": [],
                            "name": f"{inst['name']}_xw{i}",
                            "opcode": "EventSemaphore",
                            "outs": [],
                            "sync_info": {"on_update": [], "on_wait": excess[i : i + 2]},
                        })
                out.append(inst)
            blk["instructions"] = out
    return json.dumps(bir).encode()


BF16 = ml_dtypes.bfloat16
F8 = ml_dtypes.float8_e4m3
F32 = np.float32

SEQ = 32
B = 2048
H = 512
E = 512
HID = 16
NCORES = 8
BL = B // NCORES  # 256 local batch
NG = 4 * H  # 2048 gate features
BN_EPS = 1e-5

_CACHE: dict = {}


def _build_nc(repeats: int = 1):
    nc = bass.Bass()
    dt = mybir.dt
    ACTF = mybir.ActivationFunctionType
    ALU = mybir.AluOpType
    DR = mybir.MatmulPerfMode.DoubleRow

    # --- DRAM tensors (per-core inputs; weights replicated across cores) ---
    whh_d = nc.dram_tensor("whhdr", [2, 128, 2 * NG], dt.float8e4, kind="ExternalInput")
    w2t4_d = nc.dram_tensor("w2t4", [128, NG], dt.bfloat16, kind="ExternalInput")
    a1e_d = nc.dram_tensor("a1e", [2, 128], dt.float32r, kind="ExternalInput")
    id2_d = nc.dram_tensor("id2", [2, 2], dt.float32r, kind="ExternalInput")
    c1se4_d = nc.dram_tensor("c1se4", [128, 1], dt.float32, kind="ExternalInput")
    b1bf_d = nc.dram_tensor("b1bf", [4, 128, HID], dt.bfloat16, kind="ExternalInput")
    c1hp_d = nc.dram_tensor("c1hp", [HID, 1], dt.float32, kind="ExternalInput")
    hpw2_d = nc.dram_tensor("hpw2", [HID, 2], dt.bfloat16, kind="ExternalInput")
    hpb2_d = nc.dram_tensor("hpb2", [2, 1], dt.float32, kind="ExternalInput")
    zero_d = nc.dram_tensor("zero128", [128, BL], dt.bfloat16, kind="ExternalInput")
    lp0f_d = nc.dram_tensor("lp0f", [2, BL], dt.float32r, kind="ExternalInput")
    h0_d = nc.dram_tensor("h0f8", [2, 128, 2 * BL], dt.float8e4, kind="ExternalInput")
    c0_d = nc.dram_tensor("c0b", [2, 128, 2 * BL], dt.bfloat16, kind="ExternalInput")
    traj_d = nc.dram_tensor("traj", [2, SEQ, BL], dt.float32, kind="ExternalOutput")

    with tile.TileContext(nc) as tc:
        with ExitStack() as ctx:
            singles = ctx.enter_context(tc.tile_pool(name="singles", bufs=1))
            gpool = ctx.enter_context(tc.tile_pool(name="gates", bufs=2))
            tpool = ctx.enter_context(tc.tile_pool(name="temps", bufs=3))
            psg = ctx.enter_context(tc.tile_pool(name="psg", bufs=7, space="PSUM"))
            pss = ctx.enter_context(tc.tile_pool(name="pss", bufs=1, space="PSUM"))

            # persistent weights
            whh = []
            for j in range(2):
                wt = singles.tile([128, 2 * NG], dt.float8e4, tag=f"whh{j}", name=f"whh{j}")
                nc.sync.dma_start(out=wt, in_=whh_d[j, :, :])
                whh.append(wt)
            w2t4 = singles.tile([128, NG], dt.bfloat16)
            nc.sync.dma_start(out=w2t4, in_=w2t4_d[:, :])
            a1e = singles.tile([2, 128], dt.float32r)
            nc.sync.dma_start(out=a1e, in_=a1e_d[:, :])
            id2 = singles.tile([2, 2], dt.float32r)
            nc.sync.dma_start(out=id2, in_=id2_d[:, :])
            c1se4 = singles.tile([128, 1], dt.float32)
            nc.sync.dma_start(out=c1se4, in_=c1se4_d[:, :])
            b1bf = []
            for k in range(4):
                bt = singles.tile([128, HID], dt.bfloat16, tag=f"b1bf{k}", name=f"b1bf{k}")
                nc.sync.dma_start(out=bt, in_=b1bf_d[k, :, :])
                b1bf.append(bt)
            c1hp = singles.tile([HID, 1], dt.float32)
            nc.sync.dma_start(out=c1hp, in_=c1hp_d[:, :])
            hpw2 = singles.tile([HID, 2], dt.bfloat16)
            nc.sync.dma_start(out=hpw2, in_=hpw2_d[:, :])
            hpb2 = singles.tile([2, 1], dt.float32)
            nc.sync.dma_start(out=hpb2, in_=hpb2_d[:, :])
            zero128 = singles.tile([128, BL], dt.bfloat16)
            nc.sync.dma_start(out=zero128, in_=zero_d[:, :])

            # persistent state
            traj = singles.tile([2, SEQ * BL], dt.float32)
            lp0f = singles.tile([2, BL], dt.float32r)
            nc.sync.dma_start(out=lp0f, in_=lp0f_d[:, :])
            # h ping-pong: step t reads hb2[t%2], writes hb2[(t+1)%2]; fp8
            hb2, cst = {0: [], 1: []}, []
            for j in range(2):
                t_b = singles.tile([128, 2 * BL], dt.float8e4, tag=f"hbA{j}", name=f"hbA{j}")
                nc.sync.dma_start(out=t_b, in_=h0_d[j, :, :])
                hb2[0].append(t_b)
                t_b2 = singles.tile([128, 2 * BL], dt.float8e4, tag=f"hbB{j}", name=f"hbB{j}")
                hb2[1].append(t_b2)
                t_c = singles.tile([128, 2 * BL], dt.bfloat16, tag=f"c{j}", name=f"c{j}")
                nc.sync.dma_start(out=t_c, in_=c0_d[j, :, :])
                cst.append(t_c)

            whh3 = [w.rearrange("p (i c) -> p i c", i=2) for w in whh]

            pairs: dict = {}

            def dr(t, p, half, j):
                """fp8 DoubleRow h-matmul for gate m-tile m=2p+half,
                contraction pair j (h features 256j..256j+255).  The first
                matmul of the pair opens the whole-bank accumulation group;
                the group is closed by the two x-part matmuls (xmm), so all
                4 DR matmuls are independent of the lp chain and prefill."""
                ps = pairs.get((t, p))
                start = ps is None
                if start:
                    ps = psg.tile([128, 2 * BL], dt.float32, tag="gp", name=f"gp{t}_{p}")
                    pairs[(t, p)] = ps
                m = 2 * p + half
                nc.tensor.matmul(
                    ps[:, ts(half, BL)],
                    whh3[j][:, :, ts(m, 128)],
                    hb2[t % 2][j].rearrange("p (i b) -> p i b", i=2),
                    start=start, stop=False, perf_mode=DR,
                )

            def xmm(t, p, half, ru):
                """Close m-tile m=2p+half with the SE x-part (K=18, biases
                folded as hi/lo rows vs the ==1.0 rows of relu_u4).  Row
                group g=m//4 so 4 consecutive closes run concurrently."""
                ps = pairs[(t, p)]
                m = 2 * p + half
                g = m // 4
                nc.tensor.matmul(
                    ps[:, ts(half, BL)],
                    w2t4[32 * g : 32 * g + 18, ts(m, 128)],
                    ru[32 * g : 32 * g + 18, :],
                    start=False, stop=(half == 1), tile_position=(32 * g, 0),
                )

            def act(t, p):
                ps = pairs.pop((t, p))
                func = ACTF.Tanh if p in (4, 5) else ACTF.Sigmoid
                gs = gpool.tile(
                    [128, 2 * BL], dt.bfloat16, tag=f"gate{p}", name=f"gate{t}_{p}"
                )
                nc.scalar.activation(gs, ps, func)
                return gs

            def elem(t, j, gs):
                """LSTM cell update for feature-half j (bf16 on DVE at 2x,
                i*g on GpSimd); writes c in place and the NEXT ping-pong h
                buffer directly in fp8."""
                i_t, f_t, g_t, o_t = gs[0 + j], gs[2 + j], gs[4 + j], gs[6 + j]
                t_ig = tpool.tile([128, 2 * BL], dt.bfloat16, tag="tig", name=f"tig{t}_{j}")
                nc.vector.tensor_mul(t_ig, i_t, g_t)
                t_fc = tpool.tile([128, 2 * BL], dt.bfloat16, tag="tfc", name=f"tfc{t}_{j}")
                nc.vector.tensor_mul(t_fc, f_t, cst[j])
                nc.vector.tensor_add(cst[j], t_fc, t_ig)
                t_tc = tpool.tile([128, 2 * BL], dt.bfloat16, tag="ttc", name=f"ttc{t}_{j}")
                nc.scalar.activation(t_tc, cst[j], ACTF.Tanh)
                h_bf = tpool.tile([128, 2 * BL], dt.bfloat16, tag="hbf", name=f"hbf{t}_{j}", bufs=2)
                nc.vector.tensor_mul(h_bf, o_t, t_tc)
                return h_bf

            def body():
                # prologue: relu_u4 for t=0 + full prefills of pairs 0,2,4,6,1,3
                u_ps = pss.tile([128, BL], dt.float32, tag="small", name="u_0")
                nc.tensor.matmul(u_ps, a1e, lp0f, start=True, stop=True)
                ru = tpool.tile([128, BL], dt.bfloat16, tag="ru", name="ru0", bufs=2)
                nc.vector.scalar_tensor_tensor(
                    ru, u_ps, c1se4, zero128, ALU.add, ALU.max
                )
                for p in (0, 2, 4, 6, 1, 3):
                    dr(0, p, 0, 0)
                    dr(0, p, 1, 0)
                    dr(0, p, 0, 1)
                    dr(0, p, 1, 1)

                for t in range(SEQ):
                    lp_f = (
                        lp0f[:, :] if t == 0
                        else traj[:2, ts(t - 1, BL)].bitcast(dt.float32r)
                    )
                    nxt = t + 1 if t + 1 < SEQ else None
                    gs = {}

                    # ---- even pairs: x-closes first so the sigmoid stream
                    # starts as soon as relu_u4 lands ----
                    for p in (0, 2, 4, 6):
                        xmm(t, p, 0, ru)
                    for p in (0, 2, 4, 6):
                        xmm(t, p, 1, ru)
                    for p in (0, 4, 2, 6):
                        gs[p] = act(t, p)
                    hbf0 = elem(t, 0, gs)

                    # in-step DR fills for pairs 5,7 keep PE busy under the
                    # even-σ stream; then x-odds (ready: only need relu_u)
                    for p in (5, 7):
                        dr(t, p, 0, 0)
                        dr(t, p, 1, 0)
                        dr(t, p, 0, 1)
                        dr(t, p, 1, 1)
                    for p in (1, 3, 5, 7):
                        xmm(t, p, 0, ru)
                    for p in (1, 3, 5, 7):
                        xmm(t, p, 1, ru)
                    for p in (1, 5, 3, 7):
                        gs[p] = act(t, p)

                    # h0-gated PE block: v j0 + ALL j0 prefills run while the
                    # odd-σ stream / cell half 1 / lp chain proceed, keeping
                    # the PE warm (no >3.4us idle -> no HAM re-throttle)
                    nh = hb2[(t + 1) % 2]
                    v_ps = pss.tile([HID, BL], dt.float32, tag="small", name=f"v{t}")
                    for kk in (0, 1):
                        nc.tensor.matmul(
                            v_ps, b1bf[kk], hbf0[:, ts(kk, BL)],
                            start=(kk == 0), stop=False,
                        )
                    nc.vector.tensor_copy(nh[0], hbf0)
                    if nxt is not None:
                        for p in (0, 2, 4, 6, 1, 3):
                            dr(nxt, p, 0, 0)
                            dr(nxt, p, 1, 0)

                    hbf1 = elem(t, 1, gs)

                    # ---- tail: h1-gated block, then lp chain ----
                    for kk in (2, 3):
                        nc.tensor.matmul(
                            v_ps, b1bf[kk], hbf1[:, ts(kk - 2, BL)],
                            start=False, stop=(kk == 3),
                        )
                    nc.vector.tensor_copy(nh[1], hbf1)
                    if nxt is not None:
                        for p in (0, 2, 4, 6):
                            dr(nxt, p, 0, 1)
                            dr(nxt, p, 1, 1)
                    r_hp = tpool.tile([HID, BL], dt.bfloat16, tag="rhp", name=f"rhp{t}")
                    nc.vector.scalar_tensor_tensor(
                        r_hp, v_ps, c1hp, zero128[:HID, :], ALU.add, ALU.max
                    )
                    # p_ps accumulates lp (via f32r identity matmul) + HP head
                    p_ps = pss.tile([2, BL], dt.float32, tag="small", name=f"p{t}")
                    nc.tensor.matmul(p_ps, id2, lp_f, start=True, stop=False)
                    nc.tensor.matmul(p_ps, hpw2, r_hp, start=False, stop=True)
                    if nxt is not None:
                        for p in (1, 3):
                            dr(nxt, p, 0, 1)
                            dr(nxt, p, 1, 1)
                    nc.scalar.activation(
                        traj[:2, ts(t, BL)].bitcast(dt.float32r),
                        p_ps, ACTF.Sigmoid, bias=hpb2,
                    )
                    if nxt is not None:
                        u_ps = pss.tile([128, BL], dt.float32, tag="small", name=f"u{nxt}")
                        nc.tensor.matmul(
                            u_ps, a1e, traj[:2, ts(t, BL)].bitcast(dt.float32r),
                            start=True, stop=True,
                        )
                        ru = tpool.tile([128, BL], dt.bfloat16, tag="ru", name=f"ru{nxt}", bufs=2)
                        nc.vector.scalar_tensor_tensor(
                            ru, u_ps, c1se4, zero128, ALU.add, ALU.max
                        )

                if repeats > 1:
                    # restore loop-carried lp state for the next timing rep
                    nc.vector.tensor_copy(lp0f, traj[:2, ts(SEQ - 1, BL)])

            if repeats == 1:
                body()
            else:
                with tc.For_i(0, repeats, 1):
                    body()

            nc.sync.dma_start(
                out=traj_d[:, :, :].rearrange("p t b -> p (t b)"), in_=traj[:2, :]
            )
    patched = _fix_multiwait(nc.to_json_bytes())
    nc.to_json_bytes = lambda: patched
    return nc


def _pack_half(x_t):
    # [512, BL] feature-major -> [2, 128, 2*BL]: tile j holds feature-tiles
    # 2j (cols 0:BL) and 2j+1 (cols BL:2BL)
    xr = x_t.reshape(4, 128, BL)
    return np.stack(
        [np.concatenate([xr[2 * j], xr[2 * j + 1]], axis=1) for j in range(2)]
    )


def _host_prep(inputs):
    f = lambda k: np.asarray(inputs[k], dtype=np.float64)
    se_w1, se_b1 = f("se_w1"), f("se_b1")
    se_g, se_bt, se_m, se_v = f("se_g"), f("se_bt"), f("se_m"), f("se_v")
    se_w2, se_b2 = f("se_w2"), f("se_b2")
    w_ih, w_hh, b_ih, b_hh = f("w_ih"), f("w_hh"), f("b_ih"), f("b_hh")
    hp_w1, hp_b1 = f("hp_w1"), f("hp_b1")
    hp_g, hp_bt, hp_m, hp_v = f("hp_g"), f("hp_bt"), f("hp_m"), f("hp_v")
    hp_w2, hp_b2 = f("hp_w2"), f("hp_b2")

    s_se = se_g / np.sqrt(se_v + BN_EPS)
    a1 = (se_w1 * s_se[None, :]).astype(F32)  # [2, 16]
    c1_se = ((se_b1 - se_m) * s_se + se_bt).astype(F32)
    s_hp = hp_g / np.sqrt(hp_v + BN_EPS)
    b1 = (hp_w1 * s_hp[None, :]).astype(F32)  # [512, 16]
    c1_hp = ((hp_b1 - hp_m) * s_hp + hp_bt).astype(F32)

    w2t = (se_w2 @ w_ih.T).astype(F32)  # [16, 2048]
    b_eff = (b_ih + b_hh + w_ih @ se_b2).astype(F32)  # [2048]
    b_hi = b_eff.astype(BF16).astype(F32)
    b_lo = (b_eff - b_hi).astype(F32)
    w2t_ext = np.concatenate(
        [w2t, b_hi[None, :], b_lo[None, :]], axis=0
    ).astype(BF16)  # [18, 2048]

    # 4 partition-group replicas for the row-tiled x-part
    w2t4 = np.zeros((128, NG), dtype=BF16)
    a1e = np.zeros((2, 128), dtype=F32)
    c1se4 = np.zeros((128, 1), dtype=F32)
    for g in range(4):
        w2t4[32 * g : 32 * g + 18, :] = w2t_ext
        a1e[:, 32 * g : 32 * g + 16] = a1
        c1se4[32 * g : 32 * g + 16, 0] = c1_se
        c1se4[32 * g + 16 : 32 * g + 18, 0] = 1.0  # the ==1.0 bias rows

    # DoubleRow weight pairs: whhdr[j][p, i*NG+col] = w_hh.T[256j+128i+p, col]
    wT = np.ascontiguousarray(w_hh.T)  # [512, 2048]
    whhdr = wT.reshape(2, 2, 128, NG).transpose(0, 2, 1, 3).reshape(2, 128, 2 * NG)
    whhdr = whhdr.astype(F32).astype(F8)
    b1bf = b1.reshape(4, 128, HID).astype(BF16)

    rep = {
        "whhdr": np.ascontiguousarray(whhdr),
        "w2t4": np.ascontiguousarray(w2t4),
        "a1e": np.ascontiguousarray(a1e),
        "id2": np.eye(2, dtype=F32),
        "c1se4": c1se4,
        "b1bf": np.ascontiguousarray(b1bf),
        "c1hp": c1_hp.reshape(HID, 1),
        "hpw2": np.ascontiguousarray(hp_w2.astype(F32).astype(BF16)),
        "hpb2": hp_b2.astype(F32).reshape(2, 1),
        "zero128": np.zeros((128, BL), dtype=BF16),
    }

    last_pos = np.asarray(inputs["last_pos"], dtype=F32)
    h0 = np.asarray(inputs["hh"], dtype=F32)[0]
    c0 = np.asarray(inputs["ch"], dtype=F32)[0]
    in_maps = []
    for c in range(NCORES):
        rows = slice(c * BL, (c + 1) * BL)
        h0t = np.ascontiguousarray(h0[rows].T)  # [512, BL]
        c0t = np.ascontiguousarray(c0[rows].T)
        m = dict(rep)
        lp0t = np.ascontiguousarray(last_pos[rows].T)  # [2, BL]
        m["lp0f"] = lp0t
        m["h0f8"] = _pack_half(h0t).astype(F8)
        m["c0b"] = _pack_half(c0t).astype(BF16)
        in_maps.append(m)
    return in_maps


def _get_runner(repeats: int = 1):
    """Build (once) a persistent jitted SPMD runner over 8 cores."""
    key = ("runner", repeats)
    if key in _CACHE:
        return _CACHE[key]

    import jax
    from jax.sharding import Mesh, PartitionSpec, NamedSharding
    from jax.experimental.shard_map import shard_map
    from concourse import bass2jax, mybir as _mb

    nc = _build_nc(repeats)
    bass2jax.install_neuronx_cc_hook()

    partition_name = nc.partition_id_tensor.name if nc.partition_id_tensor else None
    in_names, out_names, out_avals, zero_shapes = [], [], [], []
    for alloc in nc.m.functions[0].allocations:
        if not isinstance(alloc, _mb.MemoryLocationSet):
            continue
        name = alloc.memorylocations[0].name
        if alloc.kind == "ExternalInput":
            if name != partition_name:
                in_names.append(name)
        elif alloc.kind == "ExternalOutput":
            out_names.append(name)
            shape = tuple(alloc.tensor_shape)
            dtype = _mb.dt.np(alloc.dtype)
            out_avals.append(jax.core.ShapedArray(shape, dtype))
            zero_shapes.append((shape, dtype))
    n_params = len(in_names)
    all_names = in_names + out_names
    if partition_name is not None:
        all_names = all_names + [partition_name]
    donate = tuple(range(n_params, n_params + len(out_names)))

    def _body(*args):
        operands = list(args)
        if partition_name is not None:
            operands.append(bass2jax.partition_id_tensor())
        outs = bass2jax._bass_exec_p.bind(
            *operands,
            out_avals=tuple(out_avals),
            in_names=tuple(all_names),
            out_names=tuple(out_names),
            lowering_input_output_aliases=(),
            sim_require_finite=True,
            sim_require_nnan=True,
            nc=nc,
        )
        return tuple(outs)

    devices = jax.devices()[:NCORES]
    mesh = Mesh(np.asarray(devices), ("core",))
    spec = PartitionSpec("core")
    sharded = jax.jit(
        shard_map(
            _body,
            mesh=mesh,
            in_specs=(spec,) * (n_params + len(out_names)),
            out_specs=(spec,) * len(out_names),
            check_rep=False,
        ),
        donate_argnums=donate,
        keep_unused=True,
    )
    sharding = NamedSharding(mesh, spec)

    def stage(in_maps):
        """device_put concatenated inputs once; reusable across exec() calls."""
        concat = [
            np.concatenate([np.asarray(m[name]) for m in in_maps], axis=0)
            for name in in_names
        ]
        return [jax.device_put(a, sharding) for a in concat]

    def exec_(staged):
        zeros = [
            jax.device_put(np.zeros((NCORES * s[0], *s[1:]), d), sharding)
            for s, d in zero_shapes
        ]
        outs = sharded(*staged, *zeros)
        outs = [np.asarray(o) for o in outs]
        return {
            name: outs[i].reshape(NCORES, *out_avals[i].shape)
            for i, name in enumerate(out_names)
        }

    _CACHE[key] = (stage, exec_)
    return _CACHE[key]


def kernel(**inputs) -> np.ndarray:
    stage, exec_ = _get_runner()
    staged = stage(_host_prep(inputs))
    per_core = exec_(staged)["traj"]  # [8, 2, 32, BL]
    out = per_core.transpose(2, 0, 3, 1).reshape(SEQ, B, 2)
    return np.ascontiguousarray(out.astype(np.float32))
